# revision 36
# baseline (speedup 1.0000x reference)
"""CTRNN cell (6 Euler unfolds) on 8 Trainium2 NeuronCores.

Math (per unfold, 6x):
    f     = tanh([x, s] @ W + b)
    s_new = s + 0.1 * (-s + f)  = 0.9*s + 0.1*f

Strategy (v7):
  - Data-parallel over batch: B=8192 -> 1024 rows/core, no cross-core
    communication. Host does the cheap numpy transposes/packing.
  - Transposed on-chip layout (features on SBUF partitions, batch on the
    free dim): W slices are directly the stationary lhsT, batch is the
    moving free dim.
  - ALL matmul operands fp16 (1.8e-3 rel err vs the 2e-2 gate): fp16
    streams at 1 col/cycle, halves every DMA byte, and the DVE
    tensor_tensor gets the 2x 16-bit mode.
  - Delta form: psum holds z_k = x@Wt + (10 s0)@(0.1 Wb) + sum tmp_i@wb
    across all unfolds, never restarted.  PSUM is EIGHT (128,512) tiles,
    one per (m-tile, chunk): the Tile dep tracker works at tile
    granularity, so per-(m,c) tiles let chunk c0's tanh run while chunk
    c1's matmul block is still on the PE.
  - State is never materialized: tmp_k = f_k - s_k obeys
        tmp_{k+1} = f_{k+1} + u_k,   u_k = 0.9*tmp_k - f_k  (= -s_{k+1})
    so the critical op between tanh and the next matmul round is ONE
    2x-mode tensor_tensor add per chunk.
  - Per-j input tiles.  Each k-tile j gets its own SBUF tile holding
    [wt_j | x_j] filled by ONE DMA, so round 0's first matmul gates on
    its own slice instead of the full 1.5MB x+Wt (the Tile dep tracker
    is tile-granular; a mega-tile stalled the first matmul to 14.6us).
    j0 is further split [wt0|x0c0] / [x0c1] so the first 4 matmuls gate
    on 256KB.  DMA queue order = exact need order, wb mid-stream (wb
    last cost a 3.1us PE stall + HAM re-throttle at round 0's j4).
    ONE HWDGE ring (sync/SP) for all inputs: concurrent queues share
    the 16 SDMA engines round-robin and starve the critical bytes.
  - Round 0 runs j-outer -> m -> c so one weight load serves both
    chunks; round 5 runs m-outer in both chunks so each psum completes
    early and the final tanh chain overlaps the matmul tail.
  - Host-side final combine: s6 = 0.1*f5 + 0.09*f4 - 0.81*u3, so the
    kernel just streams u3 (during round 4), f4 (during round 5) and
    f5 (tail) as fp16.  This removes ~9us of DVE combine work whose
    FIFO backlog trailed the last matmul by ~3us.  Round-5 tanh writes
    straight into per-DMA-group out tiles (c0 / c1:m0-2 / c1:m3) so
    each output DMA gates on exactly the ACTIVATEs it covers; the last
    chain is tanh(m3) -> 128KB DMA.
  - Junk warm-up matmuls (FD=512) bridge the preamble->first-bytes gap
    so the HAM clock gate is warming while the j0 DMA is in flight.
  - fp8 was evaluated and REJECTED: with the e4m3 subnormal floor fixed
    by 2^15 weight pre-scaling (descaled via the ACT input scale), a
    bit-accurate numpy sim gives 6.1e-3 max-rel for fp8 rounds 1-5 and
    2.4e-2 (over the 2e-2 gate) for the round-0 s-part; and the DVE
    cost of producing fp8 tmp tiles eats the PE win anyway.
"""

import numpy as np

UNFOLDS = 6
B, D, N = 8192, 512, 512
NCORES = 8
BC = B // NCORES          # batch rows per core
CH = 512                  # chunk = matmul moving free dim (PSUM bank)
NCH = BC // CH            # 2
P = 128
KT = D // P               # 4 k-tiles for x (and for s / tmp)
MT = N // P               # 4 m-tiles of the output dim
WX = N + BC               # cols of one combined [wt_j | x_j] tile
NJUNK = 6                 # warm-up matmuls (FD=512) spanning the load

_compiled_nc = None


def _build_nc():
    import concourse.bass as bass  # noqa: F401
    import concourse.bacc as bacc
    import concourse.tile as tile
    from concourse import mybir

    f32 = mybir.dt.float32
    f16 = mybir.dt.float16
    MULT = mybir.AluOpType.mult
    ADD = mybir.AluOpType.add
    SUB = mybir.AluOpType.subtract
    TANH = mybir.ActivationFunctionType.Tanh

    nc = bacc.Bacc("TRN2", target_bir_lowering=False, debug=False)

    wxP = nc.dram_tensor("wxP", [P, KT * WX], f16, kind="ExternalInput").ap()
    sP = nc.dram_tensor("sP", [P, KT * BC], f16, kind="ExternalInput").ap()
    wbP = nc.dram_tensor("wbP", [P, KT * N], f16, kind="ExternalInput").ap()
    bias = nc.dram_tensor("bias", [N], f32, kind="ExternalInput").ap()
    # s6 = 0.1*f5 + 0.09*f4 - 0.81*u3 is combined on the HOST: streaming
    # the three ingredients out as they appear (u3 in round 4, f4 in round
    # 5, f5 at the end) removes ~9us of DVE combine work whose FIFO
    # backlog was trailing the last matmul by ~3us.
    u3P = nc.dram_tensor("u3P", [N, BC], f16, kind="ExternalOutput").ap()
    f4P = nc.dram_tensor("f4P", [N, BC], f16, kind="ExternalOutput").ap()
    f5P = nc.dram_tensor("f5P", [N, BC], f16, kind="ExternalOutput").ap()

    with tile.TileContext(nc) as tc:
        with (
            tc.tile_pool(name="weights", bufs=1) as wpool,
            tc.tile_pool(name="data", bufs=1) as data,
            tc.tile_pool(name="fpool", bufs=2) as fpool,
            tc.tile_pool(name="tpool", bufs=2) as tpool,
            tc.tile_pool(name="upool", bufs=2) as upool,
            tc.tile_pool(name="qpool", bufs=2) as qpool,
            tc.tile_pool(name="psum", bufs=1, space="PSUM") as psump,
        ):
            junk = wpool.tile([P, CH], f16, tag="junk", name="junk")
            nc.gpsimd.memset(junk[:], 0)

            # ---- all inputs on the sync HWDGE ring, in need order ---------
            # j0 is split in four tiles so the very first real matmuls
            # (j0, c0, FD=256) gate on 192KB [wt0|x0c0a] instead of the
            # full 384KB j0 slice
            wx0a = data.tile([P, N + CH // 2], f16, tag="wx0a",
                             name="wx0a")
            wx0a2 = data.tile([P, CH // 2], f16, tag="wx0a2",
                              name="wx0a2")
            wx0b = data.tile([P, CH], f16, tag="wx0b", name="wx0b")
            wx = [None] + [data.tile([P, WX], f16, tag=f"wx{j}",
                                     name=f"wx{j}") for j in range(1, KT)]
            s_t = [data.tile([P, BC], f16, tag=f"s{j}", name=f"s{j}")
                   for j in range(KT)]
            wb_mega = wpool.tile([P, KT * N], f16, tag="wb", name="wb_mega")
            bias_sb = wpool.tile([P, MT], f32, tag="bias", name="bias_sb")

            # Need order with margin: wx_j gates round-0 block j; wb+s_j0
            # gate block j4 (the trace showed wb last -> 3.1us PE stall +
            # a HAM re-throttle).  wb rides between wx2 and wx3.
            nc.sync.dma_start(wx0a[:], wxP[:, 0:N + CH // 2])
            nc.sync.dma_start(wx0a2[:], wxP[:, N + CH // 2:N + CH])
            nc.sync.dma_start(wx0b[:], wxP[:, N + CH:WX])
            for j in range(1, 3):
                nc.sync.dma_start(wx[j][:], wxP[:, j * WX:(j + 1) * WX])
            nc.sync.dma_start(wb_mega[:], wbP[:, :])
            nc.sync.dma_start(wx[3][:], wxP[:, 3 * WX:])
            for j in range(KT):
                nc.sync.dma_start(s_t[j][:], sP[:, j * BC:(j + 1) * BC])
            nc.sync.dma_start(bias_sb[:], bias.rearrange("(m p) -> p m", p=P))

            wt = [wx0a[:, 0:N]] + [wx[j][:, 0:N] for j in range(1, KT)]
            x_sb = [None] + [wx[j][:, N:] for j in range(1, KT)]
            # j0 rhs pieces: (chunk, col-offset, width)
            x0parts = [(wx0a[:, N:], 0, 0, CH // 2),
                       (wx0a2[:], 0, CH // 2, CH // 2),
                       (wx0b[:], 1, 0, CH)]
            wb = [wb_mega[:, j * N:(j + 1) * N] for j in range(KT)]
            s_sb = [s_t[j][:] for j in range(KT)]

            # one PSUM bank per (m, chunk) so chunk streams don't serialize
            ps = [[psump.tile([P, CH], f32, tag=f"ps{m}_{c}",
                              name=f"ps{m}_{c}") for c in range(NCH)]
                  for m in range(MT)]

            # HAM warm-up: keep the PE busy while the j0 bytes stream in.
            for r in range(NJUNK):
                nc.tensor.matmul(
                    ps[r % MT][0][:],
                    lhsT=junk[:, 0:P], rhs=junk[:],
                    start=True, stop=True, skip_group_check=True,
                )

            # round 0: z = x@Wt + s10@wb.  j-outer (matches DMA arrival),
            # m next, c inner so one weight load serves both chunks.
            # j0 first: per-piece so the earliest matmuls start as soon as
            # each partial DMA lands (pieces ordered by wire arrival).
            # Only the FIRST piece of a bank may use start=True: start
            # clears the whole bank's has_written bits, so a second
            # start=True piece would wipe the first piece's columns.  The
            # second c0 piece writes its (cleared) columns with
            # start=False; the junk matmuls seeded those banks with
            # zeros, so either overwrite or accumulate semantics is safe.
            for rhs, c, co, w_ in x0parts:
                for m in range(MT):
                    nc.tensor.matmul(
                        ps[m][c][:, co:co + w_],
                        lhsT=wt[0][:, m * P:(m + 1) * P],
                        rhs=rhs,
                        start=(co == 0), stop=False,
                        skip_group_check=True,
                    )
            for j, (w, rhs) in enumerate(
                    [(wt[i], x_sb[i]) for i in range(1, KT)]
                    + [(wb[i], s_sb[i]) for i in range(KT)], start=1):
                # m-outer / c-inner so one weight load serves both chunks
                for m in range(MT):
                    for c in range(NCH):
                        cs = c * CH
                        nc.tensor.matmul(
                            ps[m][c][:],
                            lhsT=w[:, m * P:(m + 1) * P],
                            rhs=rhs[:, cs:cs + CH],
                            start=False, stop=False,
                            skip_group_check=True,
                        )

            # s0 = 0.1*s10 on the idle DVE during round 0, so unfold 0's
            # tmp0 = f0 - s0 is a 2x tensor_tensor instead of a 1x STT
            s0_t = []
            for m in range(MT):
                s0 = data.tile([P, BC], f16, tag=f"s0_{m}", name=f"s0_{m}")
                for c in range(NCH):
                    cs = c * CH
                    nc.vector.tensor_scalar_mul(
                        s0[:, cs:cs + CH], s_sb[m][:, cs:cs + CH], 0.1)
                s0_t.append(s0)

            # ---- unfolds: interleaved chunk streams -----------------------
            f_t = [None] * MT
            tmp_t = [None] * MT
            u_t = [None] * MT
            # k=5 tanh output goes straight into dedicated out tiles so
            # each output DMA gates on exactly the ACTIVATEs it covers.
            # c1 (the tail chunk) gets one tile per m and its DMAs
            # alternate sync/scalar rings, so the last 128KB DMA issues
            # on the idle scalar queue right after its own tanh instead
            # of queueing behind a 700ns descriptor-gen on sync.
            f5c0a = data.tile([P, 2 * CH], f16, tag="f5c0a", name="f5c0a")
            f5c0b = data.tile([P, 2 * CH], f16, tag="f5c0b", name="f5c0b")
            f5c1m = [data.tile([P, CH], f16, tag=f"f5c1m{m}",
                               name=f"f5c1m{m}") for m in range(MT)]
            f5v = [f5P[:, c * CH:(c + 1) * CH]
                   .rearrange("(m p) col -> p m col", p=P)
                   for c in range(NCH)]
            for k in range(UNFOLDS):
                last = k == UNFOLDS - 1
                # phase 1: tanh + critical tmp per chunk half
                fs, ts_ = [None] * MT, [None] * MT
                u_new = [None] * MT
                for c in range(NCH):
                    cs = c * CH
                    for m in range(MT):
                        if c == 0 and not last:
                            fs[m] = fpool.tile([P, BC], f16, tag=f"f{m}",
                                               name=f"f{k}_{m}")
                            ts_[m] = tpool.tile([P, BC], f16,
                                                tag=f"t{m}",
                                                name=f"t{k}_{m}")
                        if last:
                            if c == 0:
                                half = f5c0a if m < 2 else f5c0b
                                fdst = half[:, (m % 2) * CH:
                                            (m % 2 + 1) * CH]
                            else:
                                fdst = f5c1m[m][:]
                        else:
                            fdst = fs[m][:, cs:cs + CH]
                        nc.scalar.activation(
                            fdst, ps[m][c][:], TANH,
                            bias=bias_sb[:, m:m + 1], scale=1.0,
                        )
                        if last:
                            if c == 0 and m % 2 == 1:
                                half = f5c0a if m < 2 else f5c0b
                                nc.gpsimd.dma_start(
                                    f5v[0][:, m - 1:m + 1, :],
                                    half[:].rearrange(
                                        "p (m col) -> p m col", col=CH))
                            if c == 1:
                                eng = nc.sync if m % 2 == 0 else nc.scalar
                                eng.dma_start(
                                    f5P[m * P:(m + 1) * P, CH:],
                                    f5c1m[m][:])
                            continue
                        t = ts_[m]
                        if k == 0:
                            # tmp0 = f0 - s0   (2x-mode tt)
                            nc.vector.tensor_tensor(
                                t[:, cs:cs + CH], fs[m][:, cs:cs + CH],
                                s0_t[m][:, cs:cs + CH], SUB,
                            )
                        else:
                            # tmp_k = f_k + u_{k-1}   (2x-mode tt)
                            nc.vector.tensor_tensor(
                                t[:, cs:cs + CH], fs[m][:, cs:cs + CH],
                                u_t[m][:, cs:cs + CH], ADD,
                            )
                        if k == UNFOLDS - 2 and c == 1:
                            # f4 complete (both chunks): stream it out
                            nc.gpsimd.dma_start(
                                f4P[m * P:(m + 1) * P, :], fs[m][:])
                if last:
                    break
                f_t, tmp_t = fs, ts_
                # phase 2 (k<4): u pass full width on DVE, off the critical
                # path: q = 0.9*tmp (4x tensor_scalar), u = q - f (2x tt).
                # GpSimd is NOT used: a concurrent Pool elementwise op
                # steals the shared SBUF port and slows DVE ops ~3.3x.
                for m in range(MT) if k < UNFOLDS - 2 else ():
                    u = upool.tile([P, BC], f16, tag=f"u{m}",
                                   name=f"u{k}_{m}")
                    q = qpool.tile([P, BC], f16, tag=f"qw{m}",
                                   name=f"qw{k}_{m}")
                    nc.vector.tensor_scalar_mul(q[:], tmp_t[m][:], 0.9)
                    nc.vector.tensor_tensor(u[:], q[:], f_t[m][:], SUB)
                    u_new[m] = u
                    if k == UNFOLDS - 3:
                        # u3 feeds the host-side combine: out while r4 runs
                        nc.gpsimd.dma_start(
                            u3P[m * P:(m + 1) * P, :], u[:])
                for m in range(MT):
                    u_t[m] = u_new[m]
                # phase 3: next matmul round, chunk-interleaved. The last
                # round (R5) runs m-outer in BOTH chunks: its tmp deps are
                # long ready, and each psum tile completing early lets the
                # final tanh chain overlap the matmul tail instead of
                # trailing it.
                for c in range(NCH):
                    cs = c * CH
                    m_outer = k == UNFOLDS - 2
                    order = ([(j, m) for m in range(MT) for j in range(KT)]
                             if m_outer else
                             [(j, m) for j in range(KT) for m in range(MT)])
                    for j, m in order:
                        nc.tensor.matmul(
                            ps[m][c][:],
                            lhsT=wb[j][:, m * P:(m + 1) * P],
                            rhs=tmp_t[j][:, cs:cs + CH],
                            start=False,
                            stop=(k == UNFOLDS - 2 and j == KT - 1),
                            skip_group_check=True,
                        )

    nc.compile()
    return nc


def _get_nc():
    global _compiled_nc
    if _compiled_nc is None:
        _compiled_nc = _build_nc()
    return _compiled_nc


def make_in_maps(x, s, W, b):
    """Shard + pack host-side: everything fp16, (128, k*C) layouts with
    k-tiles side by side so per-partition DMA runs are >=1KB contiguous."""
    xT = np.ascontiguousarray(x.T)            # (D, B) f32
    sT = np.ascontiguousarray(10.0 * s.T)     # (N, B) f32, pre-scaled
    wt = W[:D].reshape(KT, P, N).transpose(1, 0, 2)          # (P, KT, N)
    wb = np.ascontiguousarray(
        (0.1 * W[D:]).reshape(KT, P, N).transpose(1, 0, 2).reshape(P, -1)
    ).astype(np.float16)
    in_maps = []
    for c in range(NCORES):
        sl = slice(c * BC, (c + 1) * BC)
        xs = xT[:, sl].reshape(KT, P, BC).transpose(1, 0, 2)  # (P, KT, BC)
        wxs = np.concatenate([wt, xs], axis=2).reshape(P, -1)
        ss = sT[:, sl].reshape(KT, P, BC).transpose(1, 0, 2).reshape(P, -1)
        in_maps.append({
            "wxP": np.ascontiguousarray(wxs).astype(np.float16),
            "sP": np.ascontiguousarray(ss).astype(np.float16),
            "wbP": wb,
            "bias": np.ascontiguousarray(b.astype(np.float32)),
        })
    return in_maps


def kernel(**inputs):
    from concourse.bass_utils import run_bass_kernel_spmd

    x = np.asarray(inputs["inputs"], dtype=np.float32)
    s = np.asarray(inputs["state"], dtype=np.float32)
    W = np.ascontiguousarray(np.asarray(inputs["W"], dtype=np.float32))
    b = np.ascontiguousarray(np.asarray(inputs["bias"], dtype=np.float32))

    in_maps = make_in_maps(x, s, W, b)
    nc = _get_nc()
    res = run_bass_kernel_spmd(nc, in_maps, list(range(NCORES))).results
    u3 = np.concatenate([res[c]["u3P"] for c in range(NCORES)], axis=1)
    f4 = np.concatenate([res[c]["f4P"] for c in range(NCORES)], axis=1)
    f5 = np.concatenate([res[c]["f5P"] for c in range(NCORES)], axis=1)
    # s6 = 0.9*s5 + 0.1*f5,  s5 = 0.1*f4 - 0.9*u3  (u_k = -s_{k+1})
    outT = (0.1 * f5.astype(np.float32) + 0.09 * f4.astype(np.float32)
            - 0.81 * u3.astype(np.float32))
    out = np.ascontiguousarray(outT.T)
    return (out, out)


# revision 40
# speedup vs baseline: 1.1739x; 1.1739x over previous
"""CTRNN cell (6 Euler unfolds) on 8 Trainium2 NeuronCores.

Math (per unfold, 6x):
    f     = tanh([x, s] @ W + b)
    s_new = s + 0.1 * (-s + f)  = 0.9*s + 0.1*f

Strategy (v7):
  - Data-parallel over batch: B=8192 -> 1024 rows/core, no cross-core
    communication. Host does the cheap numpy transposes/packing.
  - Transposed on-chip layout (features on SBUF partitions, batch on the
    free dim): W slices are directly the stationary lhsT, batch is the
    moving free dim.
  - ALL matmul operands fp16 (1.8e-3 rel err vs the 2e-2 gate): fp16
    streams at 1 col/cycle, halves every DMA byte, and the DVE
    tensor_tensor gets the 2x 16-bit mode.
  - Delta form: psum holds z_k = x@Wt + (10 s0)@(0.1 Wb) + sum tmp_i@wb
    across all unfolds, never restarted.  PSUM is EIGHT (128,512) tiles,
    one per (m-tile, chunk): the Tile dep tracker works at tile
    granularity, so per-(m,c) tiles let chunk c0's tanh run while chunk
    c1's matmul block is still on the PE.
  - State is never materialized: tmp_k = f_k - s_k obeys
        tmp_{k+1} = f_{k+1} + u_k,   u_k = 0.9*tmp_k - f_k  (= -s_{k+1})
    so the critical op between tanh and the next matmul round is ONE
    2x-mode tensor_tensor add per chunk.
  - Per-j input tiles.  Each k-tile j gets its own SBUF tile holding
    [wt_j | x_j] filled by ONE DMA, so round 0's first matmul gates on
    its own slice instead of the full 1.5MB x+Wt (the Tile dep tracker
    is tile-granular; a mega-tile stalled the first matmul to 14.6us).
    j0 is further split [wt0|x0c0] / [x0c1] so the first 4 matmuls gate
    on 256KB.  DMA queue order = exact need order, wb mid-stream (wb
    last cost a 3.1us PE stall + HAM re-throttle at round 0's j4).
    ONE HWDGE ring (sync/SP) for all inputs: concurrent queues share
    the 16 SDMA engines round-robin and starve the critical bytes.
  - Round 0 runs j-outer -> m -> c so one weight load serves both
    chunks; round 5 runs m-outer in both chunks so each psum completes
    early and the final tanh chain overlaps the matmul tail.
  - Host-side final combine: s6 = 0.1*f5 + 0.09*f4 - 0.81*u3, so the
    kernel just streams u3 (during round 4), f4 (during round 5) and
    f5 (tail) as fp16.  This removes ~9us of DVE combine work whose
    FIFO backlog trailed the last matmul by ~3us.  Round-5 tanh writes
    straight into per-DMA-group out tiles (c0 / c1:m0-2 / c1:m3) so
    each output DMA gates on exactly the ACTIVATEs it covers; the last
    chain is tanh(m3) -> 128KB DMA.
  - Junk warm-up matmuls (FD=512) bridge the preamble->first-bytes gap
    so the HAM clock gate is warming while the j0 DMA is in flight.
  - fp8 was evaluated and REJECTED: with the e4m3 subnormal floor fixed
    by 2^15 weight pre-scaling (descaled via the ACT input scale), a
    bit-accurate numpy sim gives 6.1e-3 max-rel for fp8 rounds 1-5 and
    2.4e-2 (over the 2e-2 gate) for the round-0 s-part; and the DVE
    cost of producing fp8 tmp tiles eats the PE win anyway.
"""

import numpy as np

UNFOLDS = 6
B, D, N = 8192, 512, 512
NCORES = 8
BC = B // NCORES          # batch rows per core
CH = 512                  # chunk = matmul moving free dim (PSUM bank)
NCH = BC // CH            # 2
P = 128
KT = D // P               # 4 k-tiles for x (and for s / tmp)
MT = N // P               # 4 m-tiles of the output dim
WX = N + BC               # cols of one combined [wt_j | x_j] tile
NJUNK = 28                # warm-up matmuls (FD=128) spanning the load

_compiled_nc = None


def _build_nc():
    import concourse.bass as bass  # noqa: F401
    import concourse.bacc as bacc
    import concourse.tile as tile
    from concourse import mybir

    f32 = mybir.dt.float32
    f16 = mybir.dt.float16
    MULT = mybir.AluOpType.mult
    ADD = mybir.AluOpType.add
    SUB = mybir.AluOpType.subtract
    TANH = mybir.ActivationFunctionType.Tanh

    nc = bacc.Bacc("TRN2", target_bir_lowering=False, debug=False)

    wxP = nc.dram_tensor("wxP", [P, KT * WX], f16, kind="ExternalInput").ap()
    sP = nc.dram_tensor("sP", [P, KT * BC], f16, kind="ExternalInput").ap()
    wbP = nc.dram_tensor("wbP", [P, KT * N], f16, kind="ExternalInput").ap()
    bias = nc.dram_tensor("bias", [N], f32, kind="ExternalInput").ap()
    # s6 = 0.1*f5 + 0.09*f4 - 0.81*u3 is combined on the HOST: streaming
    # the three ingredients out as they appear (u3 in round 4, f4 in round
    # 5, f5 at the end) removes ~9us of DVE combine work whose FIFO
    # backlog was trailing the last matmul by ~3us.
    u3P = nc.dram_tensor("u3P", [N, BC], f16, kind="ExternalOutput").ap()
    f4P = nc.dram_tensor("f4P", [N, BC], f16, kind="ExternalOutput").ap()
    f5P = nc.dram_tensor("f5P", [N, BC], f16, kind="ExternalOutput").ap()

    with tile.TileContext(nc) as tc:
        with (
            tc.tile_pool(name="weights", bufs=1) as wpool,
            tc.tile_pool(name="data", bufs=1) as data,
            tc.tile_pool(name="fpool", bufs=2) as fpool,
            tc.tile_pool(name="tpool", bufs=2) as tpool,
            tc.tile_pool(name="upool", bufs=2) as upool,
            tc.tile_pool(name="qpool", bufs=2) as qpool,
            tc.tile_pool(name="psum", bufs=1, space="PSUM") as psump,
        ):
            # Small junk tile: its memset gates the first warm-up matmul
            # (and so the HAM clock-gate ramp); (128,128) memsets in
            # ~160ns vs 627ns for (128,512)
            junk = wpool.tile([P, P], f16, tag="junk", name="junk")
            nc.gpsimd.memset(junk[:], 0)

            # ---- all inputs on the sync HWDGE ring, in need order ---------
            # j0 is split in two tiles so the very first real matmuls
            # (j0, c0) gate on 256KB [wt0|x0c0] instead of the full 384KB
            wx0a = data.tile([P, N + CH], f16, tag="wx0a", name="wx0a")
            wx0b = data.tile([P, CH], f16, tag="wx0b", name="wx0b")
            wx = [None] + [data.tile([P, WX], f16, tag=f"wx{j}",
                                     name=f"wx{j}") for j in range(1, KT)]
            s_t = [data.tile([P, BC], f16, tag=f"s{j}", name=f"s{j}")
                   for j in range(KT)]
            wb_mega = wpool.tile([P, KT * N], f16, tag="wb", name="wb_mega")
            bias_sb = wpool.tile([P, MT], f32, tag="bias", name="bias_sb")

            # Need order with margin: wx_j gates round-0 block j; wb+s_j0
            # gate block j4 (the trace showed wb last -> 3.1us PE stall +
            # a HAM re-throttle).  wb rides between wx2 and wx3.
            nc.sync.dma_start(wx0a[:], wxP[:, 0:N + CH])
            nc.sync.dma_start(wx0b[:], wxP[:, N + CH:WX])
            for j in range(1, 3):
                nc.sync.dma_start(wx[j][:], wxP[:, j * WX:(j + 1) * WX])
            nc.sync.dma_start(wb_mega[:], wbP[:, :])
            nc.sync.dma_start(wx[3][:], wxP[:, 3 * WX:])
            for j in range(KT):
                nc.sync.dma_start(s_t[j][:], sP[:, j * BC:(j + 1) * BC])
            nc.sync.dma_start(bias_sb[:], bias.rearrange("(m p) -> p m", p=P))

            wt = [wx0a[:, 0:N]] + [wx[j][:, 0:N] for j in range(1, KT)]
            x_sb = [None] + [wx[j][:, N:] for j in range(1, KT)]
            x0c = [wx0a[:, N:], wx0b[:]]   # j0 rhs per chunk
            wb = [wb_mega[:, j * N:(j + 1) * N] for j in range(KT)]
            s_sb = [s_t[j][:] for j in range(KT)]

            # one PSUM bank per (m, chunk) so chunk streams don't serialize
            ps = [[psump.tile([P, CH], f32, tag=f"ps{m}_{c}",
                              name=f"ps{m}_{c}") for c in range(NCH)]
                  for m in range(MT)]

            # HAM warm-up: keep the PE busy while the j0 bytes stream in.
            for r in range(NJUNK):
                nc.tensor.matmul(
                    ps[r % MT][0][:, 0:P],
                    lhsT=junk[:], rhs=junk[:],
                    start=True, stop=True, skip_group_check=True,
                )

            # round 0: z = x@Wt + s10@wb.  j-outer (matches DMA arrival),
            # m next, c inner so one weight load serves both chunks.
            for j, (w, rhs) in enumerate(
                    [(wt[i], x_sb[i]) for i in range(KT)]
                    + [(wb[i], s_sb[i]) for i in range(KT)]):
                # j0: c-outer so the c0 block runs while x0's c1 half is
                # still on the wire; elsewhere m-outer/c-inner so one
                # weight load serves both chunks.
                order = ([(m, c) for c in range(NCH) for m in range(MT)]
                         if j == 0 else
                         [(m, c) for m in range(MT) for c in range(NCH)])
                for m, c in order:
                    cs = c * CH
                    nc.tensor.matmul(
                        ps[m][c][:],
                        lhsT=w[:, m * P:(m + 1) * P],
                        rhs=(x0c[c] if j == 0
                             else rhs[:, cs:cs + CH]),
                        start=(j == 0), stop=False,
                        skip_group_check=True,
                    )

            # s0 = 0.1*s10 on the idle DVE during round 0, so unfold 0's
            # tmp0 = f0 - s0 is a 2x tensor_tensor instead of a 1x STT
            s0_t = []
            for m in range(MT):
                s0 = data.tile([P, BC], f16, tag=f"s0_{m}", name=f"s0_{m}")
                for c in range(NCH):
                    cs = c * CH
                    nc.vector.tensor_scalar_mul(
                        s0[:, cs:cs + CH], s_sb[m][:, cs:cs + CH], 0.1)
                s0_t.append(s0)

            # ---- unfolds: interleaved chunk streams -----------------------
            f_t = [None] * MT
            tmp_t = [None] * MT
            u_t = [None] * MT
            # k=5 tanh output goes straight into dedicated out tiles so
            # each output DMA gates on exactly the ACTIVATEs it covers.
            # c1 (the tail chunk) gets one tile per m and its DMAs
            # alternate sync/scalar rings, so the last 128KB DMA issues
            # on the idle scalar queue right after its own tanh instead
            # of queueing behind a 700ns descriptor-gen on sync.
            f5c0a = data.tile([P, 2 * CH], f16, tag="f5c0a", name="f5c0a")
            f5c0b = data.tile([P, 2 * CH], f16, tag="f5c0b", name="f5c0b")
            f5c1m = [data.tile([P, CH], f16, tag=f"f5c1m{m}",
                               name=f"f5c1m{m}") for m in range(MT)]
            f5v = [f5P[:, c * CH:(c + 1) * CH]
                   .rearrange("(m p) col -> p m col", p=P)
                   for c in range(NCH)]
            for k in range(UNFOLDS):
                last = k == UNFOLDS - 1
                # phase 1: tanh + critical tmp per chunk half
                fs, ts_ = [None] * MT, [None] * MT
                u_new = [None] * MT
                for c in range(NCH):
                    cs = c * CH
                    for m in range(MT):
                        if c == 0 and not last:
                            fs[m] = fpool.tile([P, BC], f16, tag=f"f{m}",
                                               name=f"f{k}_{m}")
                            ts_[m] = tpool.tile([P, BC], f16,
                                                tag=f"t{m}",
                                                name=f"t{k}_{m}")
                        if last:
                            if c == 0:
                                half = f5c0a if m < 2 else f5c0b
                                fdst = half[:, (m % 2) * CH:
                                            (m % 2 + 1) * CH]
                            else:
                                fdst = f5c1m[m][:]
                        else:
                            fdst = fs[m][:, cs:cs + CH]
                        nc.scalar.activation(
                            fdst, ps[m][c][:], TANH,
                            bias=bias_sb[:, m:m + 1], scale=1.0,
                        )
                        if last:
                            if c == 0 and m % 2 == 1:
                                half = f5c0a if m < 2 else f5c0b
                                nc.gpsimd.dma_start(
                                    f5v[0][:, m - 1:m + 1, :],
                                    half[:].rearrange(
                                        "p (m col) -> p m col", col=CH))
                            if c == 1:
                                eng = nc.sync if m % 2 == 0 else nc.scalar
                                eng.dma_start(
                                    f5P[m * P:(m + 1) * P, CH:],
                                    f5c1m[m][:])
                            continue
                        t = ts_[m]
                        if k == 0:
                            # tmp0 = f0 - s0   (2x-mode tt)
                            nc.vector.tensor_tensor(
                                t[:, cs:cs + CH], fs[m][:, cs:cs + CH],
                                s0_t[m][:, cs:cs + CH], SUB,
                            )
                        else:
                            # tmp_k = f_k + u_{k-1}   (2x-mode tt)
                            nc.vector.tensor_tensor(
                                t[:, cs:cs + CH], fs[m][:, cs:cs + CH],
                                u_t[m][:, cs:cs + CH], ADD,
                            )
                        if k == UNFOLDS - 2 and c == 1:
                            # f4 complete (both chunks): stream it out
                            nc.gpsimd.dma_start(
                                f4P[m * P:(m + 1) * P, :], fs[m][:])
                if last:
                    break
                f_t, tmp_t = fs, ts_
                # phase 2 (k<4): u pass full width on DVE, off the critical
                # path: q = 0.9*tmp (4x tensor_scalar), u = q - f (2x tt).
                # GpSimd is NOT used: a concurrent Pool elementwise op
                # steals the shared SBUF port and slows DVE ops ~3.3x.
                for m in range(MT) if k < UNFOLDS - 2 else ():
                    u = upool.tile([P, BC], f16, tag=f"u{m}",
                                   name=f"u{k}_{m}")
                    q = qpool.tile([P, BC], f16, tag=f"qw{m}",
                                   name=f"qw{k}_{m}")
                    nc.vector.tensor_scalar_mul(q[:], tmp_t[m][:], 0.9)
                    nc.vector.tensor_tensor(u[:], q[:], f_t[m][:], SUB)
                    u_new[m] = u
                    if k == UNFOLDS - 3:
                        # u3 feeds the host-side combine: out while r4 runs
                        nc.gpsimd.dma_start(
                            u3P[m * P:(m + 1) * P, :], u[:])
                for m in range(MT):
                    u_t[m] = u_new[m]
                # phase 3: next matmul round, chunk-interleaved. The last
                # round (R5) runs m-outer in BOTH chunks: its tmp deps are
                # long ready, and each psum tile completing early lets the
                # final tanh chain overlap the matmul tail instead of
                # trailing it.
                for c in range(NCH):
                    cs = c * CH
                    m_outer = k == UNFOLDS - 2
                    order = ([(j, m) for m in range(MT) for j in range(KT)]
                             if m_outer else
                             [(j, m) for j in range(KT) for m in range(MT)])
                    for j, m in order:
                        nc.tensor.matmul(
                            ps[m][c][:],
                            lhsT=wb[j][:, m * P:(m + 1) * P],
                            rhs=tmp_t[j][:, cs:cs + CH],
                            start=False,
                            stop=(k == UNFOLDS - 2 and j == KT - 1),
                            skip_group_check=True,
                        )

    nc.compile()
    return nc


def _get_nc():
    global _compiled_nc
    if _compiled_nc is None:
        _compiled_nc = _build_nc()
    return _compiled_nc


def make_in_maps(x, s, W, b):
    """Shard + pack host-side: everything fp16, (128, k*C) layouts with
    k-tiles side by side so per-partition DMA runs are >=1KB contiguous."""
    xT = np.ascontiguousarray(x.T)            # (D, B) f32
    sT = np.ascontiguousarray(10.0 * s.T)     # (N, B) f32, pre-scaled
    wt = W[:D].reshape(KT, P, N).transpose(1, 0, 2)          # (P, KT, N)
    wb = np.ascontiguousarray(
        (0.1 * W[D:]).reshape(KT, P, N).transpose(1, 0, 2).reshape(P, -1)
    ).astype(np.float16)
    in_maps = []
    for c in range(NCORES):
        sl = slice(c * BC, (c + 1) * BC)
        xs = xT[:, sl].reshape(KT, P, BC).transpose(1, 0, 2)  # (P, KT, BC)
        wxs = np.concatenate([wt, xs], axis=2).reshape(P, -1)
        ss = sT[:, sl].reshape(KT, P, BC).transpose(1, 0, 2).reshape(P, -1)
        in_maps.append({
            "wxP": np.ascontiguousarray(wxs).astype(np.float16),
            "sP": np.ascontiguousarray(ss).astype(np.float16),
            "wbP": wb,
            "bias": np.ascontiguousarray(b.astype(np.float32)),
        })
    return in_maps


def kernel(**inputs):
    from concourse.bass_utils import run_bass_kernel_spmd

    x = np.asarray(inputs["inputs"], dtype=np.float32)
    s = np.asarray(inputs["state"], dtype=np.float32)
    W = np.ascontiguousarray(np.asarray(inputs["W"], dtype=np.float32))
    b = np.ascontiguousarray(np.asarray(inputs["bias"], dtype=np.float32))

    in_maps = make_in_maps(x, s, W, b)
    nc = _get_nc()
    res = run_bass_kernel_spmd(nc, in_maps, list(range(NCORES))).results
    u3 = np.concatenate([res[c]["u3P"] for c in range(NCORES)], axis=1)
    f4 = np.concatenate([res[c]["f4P"] for c in range(NCORES)], axis=1)
    f5 = np.concatenate([res[c]["f5P"] for c in range(NCORES)], axis=1)
    # s6 = 0.9*s5 + 0.1*f5,  s5 = 0.1*f4 - 0.9*u3  (u_k = -s_{k+1})
    outT = (0.1 * f5.astype(np.float32) + 0.09 * f4.astype(np.float32)
            - 0.81 * u3.astype(np.float32))
    out = np.ascontiguousarray(outT.T)
    return (out, out)
